# revision 4
# baseline (speedup 1.0000x reference)
"""Trainium2 Bass kernel for nn_CRFTModule (moe_routing).

Pure data parallel over batch: 8 cores, one batch row (4096 tokens) each.

Math per core (batch b, S=4096 tokens, H=1024):
  z      = gelu(x @ W1 + b1) @ W2 + b2              (critical-path detector)
  mask   = z > logit(0.7)                            (compare in logit space)
  aw     = softmax(x[last] @ sel_w + sel_b)          (adapter selector, 4-way)
  t      = gelu(x @ Dcat + db)                       (all 4 down-projs, [S,32])
  wm     = 0.3 * mask * (sum_a aw[a] (t_a @ up_w[a] + up_b[a]))
  out    = x + wm

v2 design notes (cost-model driven):
  - PE matmul cost = out free-size x 0.4167ns x cycles_per_row; weight loads
    are free.  So: down-proj runs in token-partition orientation (out [tok,32],
    ap=32), detector mm2 in natural orientation (out [tok,1], ap=1), and the
    up-proj uses fp8e4 DoubleRow (0.5 cycles/row) with the adapter softmax
    weights and 0.3*16 scale folded into the up matrix (1/16 folded into the
    mask value to keep fp8 operands in normal range).
  - x is cast f32->f16 once (split DVE/ACT), transposed mostly on PE
    (128x128 blocks -> psum -> DVE 2x copies) with one half-chunk per tile
    offloaded to the DMA xbar transpose to balance PE vs DMA.
  - output is written f16 (rounding ~6e-5 rel) halving write traffic; host
    upcasts to f32.
  - detector mm1 stays f16: the threshold compare needs ~4e-3 accuracy on z.
"""
import math

import numpy as np

import concourse.bacc as bacc
import concourse.mybir as mybir
from concourse.bass import ts
from concourse.tile import TileContext
from concourse.bass_utils import run_bass_kernel_spmd

dt = mybir.dt
AF = mybir.ActivationFunctionType
ALU = mybir.AluOpType
PM = mybir.MatmulPerfMode

B, S, H = 8, 4096, 1024
A_DIM, N_ADAPT = 8, 4
PD = H // 2              # 512 detector hidden dim
SCALE = 0.3
THRESH = 0.7
N_CORES = 8
T = 512                  # tokens per tile
N_TILES = S // T         # 8
KUP = N_ADAPT * A_DIM + N_ADAPT  # 36
LAM = 16.0               # fp8 range scale folded into uw, undone in mask

# f16 const blob columns: w1(8*512) | dcat(8*32) | w2(4) | idh(128) | dbrow(32)
_F16_COLS = 8 * PD + 8 * 32 + 4 + 128 + 32
# f32 const blob columns: b1(4) thr(1) selw(32) selb(1) o14(4) i4(4) e36(36)
_F32_COLS = 82
_XBAR_HALVES = 1         # half-j chunks per tile transposed via DMA xbar


def _build():
    nc = bacc.Bacc("TRN2", target_bir_lowering=False, debug=False)

    x = nc.declare_dram_parameter("x", [S, H], dt.float32, isOutput=False)
    fb16 = nc.declare_dram_parameter("fb16", [128, _F16_COLS], dt.float16, isOutput=False)
    fb32 = nc.declare_dram_parameter("fb32", [128, _F32_COLS], dt.float32, isOutput=False)
    u36 = nc.declare_dram_parameter("u36", [18, 2, H], dt.float16, isOutput=False)
    out = nc.declare_dram_parameter("out", [S, H], dt.float16, isOutput=True)

    with TileContext(nc) as tc:
        with (
            tc.tile_pool(name="consts", bufs=1) as cp,
            tc.tile_pool(name="work", bufs=2) as wp,
            tc.tile_pool(name="psum", bufs=2, space="PSUM") as pp,
        ):
            # prefetch tile 0 activations (two halves) before const loads
            X0 = wp.tile([128, 4, H], dt.float32, tag="X", name="Xpre", bufs=3)
            for h in range(2):
                nc.sync.dma_start(
                    out=X0[:, 2 * h : 2 * h + 2, :],
                    in_=x[h * 256 : (h + 1) * 256, :].rearrange(
                        "(j p) h -> p j h", p=128
                    ),
                )

            # ---- constants ----
            c16 = cp.tile([128, _F16_COLS], dt.float16, tag="c16")
            _ID0 = 8 * PD + 260
            # idh + dcat + w2 + dbrow first (transposes + down-proj need them),
            # then w1 in two chunks so mm1 can start before the full blob lands
            nc.sync.dma_start(out=c16[:, 8 * PD : _F16_COLS], in_=fb16[:, 8 * PD : _F16_COLS])
            nc.sync.dma_start(out=c16[:, 0 : 4 * PD], in_=fb16[:, 0 : 4 * PD])
            nc.sync.dma_start(out=c16[:, 4 * PD : 8 * PD], in_=fb16[:, 4 * PD : 8 * PD])
            c32 = cp.tile([128, _F32_COLS], dt.float32, tag="c32")
            nc.sync.dma_start(out=c32[:], in_=fb32[:])
            u_sb = cp.tile([18, 2, H], dt.float16, tag="u36")
            nc.sync.dma_start(out=u_sb[:], in_=u36[:])
            xlast0 = cp.tile([128, 8], dt.float32, tag="xlast")
            nc.sync.dma_start(
                out=xlast0[:],
                in_=x.rearrange("s (c p) -> p s c", p=128)[:, S - 1, :],
            )

            w1v = c16[:, 0 : 8 * PD].rearrange("p (c n) -> p c n", c=8)
            dcv = c16[:, 8 * PD : 8 * PD + 256].rearrange("p (c n) -> p c n", c=8)
            w2v = c16[:, 8 * PD + 256 : 8 * PD + 260]
            idh = c16[:, 8 * PD + 260 : 8 * PD + 388]
            dbrow = c16[0:1, 8 * PD + 388 : 8 * PD + 420]
            b1v = c32[:, 0:4]
            thrv = c32[:, 4:5]
            selwv = c32[:, 5:37].rearrange("p (c a) -> p c a", c=8)
            selbv = c32[0:4, 37:38]
            o14v = c32[0:1, 38:42]
            i4v = c32[0:4, 42:46]
            e36v = c32[0:4, 46:82]

            # ones row for the down-proj bias matmul
            ones1 = cp.tile([1, T], dt.float16, tag="ones1")
            nc.gpsimd.memset(ones1[:], 1.0)

            # dummy ACT op so the gelu table set loads during startup DMAs
            dummy = cp.tile([1, 1], dt.float16, tag="dummy")
            nc.scalar.copy(dummy[:], idh[0:1, 0:1])

            # ---- adapter selector (once per core) ----
            ps_sel = pp.tile([4, 1], dt.float32, tag="small", bufs=2)
            for c in range(8):
                nc.tensor.matmul(
                    ps_sel[:], selwv[:, c, :], xlast0[:, c : c + 1],
                    start=(c == 0), stop=(c == 7),
                )
            # t = tanh((z + sel_b)/2)  -> exp(z+sel_b) = (1+t)/(1-t)
            t4 = cp.tile([4, 1], dt.float32, tag="t4")
            nc.scalar.activation(t4[:], ps_sel[:], AF.Tanh, bias=selbv, scale=0.5)
            num4 = cp.tile([4, 1], dt.float32, tag="num4")
            nc.vector.tensor_scalar(num4[:], t4[:], 1.0, None, ALU.add)
            den4 = cp.tile([4, 1], dt.float32, tag="den4")
            nc.vector.tensor_scalar(den4[:], t4[:], -1.0, 1.0, ALU.mult, ALU.add)
            rden4 = cp.tile([4, 1], dt.float32, tag="rden4")
            nc.vector.reciprocal(rden4[:], den4[:])
            e4 = cp.tile([4, 1], dt.float32, tag="e4")
            nc.vector.tensor_mul(e4[:], num4[:], rden4[:])
            ps_et = pp.tile([1, 4], dt.float32, tag="small", bufs=2)
            nc.tensor.matmul(ps_et[:], e4[:], i4v, start=True, stop=True)
            ssum = cp.tile([1, 1], dt.float32, tag="ssum")
            nc.vector.reduce_sum(ssum[:], ps_et[:], axis=mybir.AxisListType.X)
            rsum = cp.tile([1, 1], dt.float32, tag="rsum")
            nc.vector.reciprocal(rsum[:], ssum[:])
            ps_rs = pp.tile([4, 1], dt.float32, tag="small", bufs=2)
            nc.tensor.matmul(ps_rs[:], o14v, rsum[:], start=True, stop=True)
            w4 = cp.tile([4, 1], dt.float32, tag="w4")
            nc.vector.tensor_tensor(w4[:], e4[:], ps_rs[:], ALU.mult)
            # wv in DoubleRow layout [18, 2]: rows 0:18 / 18:36 of the 36-vec
            ps_wv = pp.tile([18, 2], dt.float32, tag="small", bufs=2)
            for half in range(2):
                nc.tensor.matmul(
                    ps_wv[:, half : half + 1], e36v[:, 18 * half : 18 * half + 18],
                    w4[:], start=True, stop=True,
                )
            wv_sb = cp.tile([18, 2], dt.float32, tag="wv")
            nc.scalar.copy(wv_sb[:], ps_wv[:])
            # fold adapter weights into the fp8 up matrix
            uw = cp.tile([18, 2, H], dt.float8e4, tag="uw")
            for half in range(2):
                nc.vector.tensor_scalar(
                    uw[:, half, :], u_sb[:, half, :],
                    wv_sb[:, half : half + 1], None, ALU.mult,
                )

            # ---- main loop over token tiles ----
            for i in range(N_TILES):
                if i == 0:
                    Xp = X0
                else:
                    Xp = wp.tile([128, 4, H], dt.float32, tag="X", name=f"X{i}", bufs=3)
                    nc.sync.dma_start(
                        out=Xp[:],
                        in_=x[i * T : (i + 1) * T, :].rearrange(
                            "(j p) h -> p j h", p=128
                        ),
                    )

                # cast f32 -> f16, one op per 128-token chunk (Pool)
                Xh = wp.tile([128, 4, H], dt.float16, tag="Xh", name=f"Xh{i}", bufs=2)
                for j in range(4):
                    nc.gpsimd.tensor_copy(Xh[:, j, :], Xp[:, j, :])

                # transpose x: XT[:, c, j, :] = x[j*128:(j+1)*128, c*128:(c+1)*128]^T
                XT = wp.tile([128, 8, 4, 128], dt.float16, tag="XT", name=f"XT{i}", bufs=2)
                # xbar route: first half of j=0
                nc.sync.dma_start_transpose(XT[:, 0:4, 0, :], Xh[:, 0, 0:PD])
                # PE route for the rest; psum->sbuf copies split DVE/ACT
                for j in range(4):
                    c0 = 4 if j == 0 else 0
                    ncx = 8 - c0
                    ptp = pp.tile(
                        [128, ncx, 128], dt.float16, tag="pt", name=f"pt{i}_{j}", bufs=1,
                    )
                    for c in range(c0, 8):
                        nc.tensor.transpose(
                            ptp[:, c - c0, :], Xh[:, j, ts(c, 128)], idh
                        )
                    if j < 2:
                        nc.vector.tensor_copy(XT[:, c0:8, j, :], ptp[:])
                    else:
                        nc.scalar.copy(XT[:, c0:8, j, :], ptp[:])

                def xtc(c):
                    return XT[:, c, :, :]

                # detector mm1 + gelu (transposed orientation: k on partitions)
                Hs = []
                for q in range(4):
                    ps_h = pp.tile([128, T], dt.float32, tag="h", name=f"psh{i}_{q}")
                    for c in range(8):
                        nc.tensor.matmul(
                            ps_h[:], w1v[:, c, ts(q, 128)], xtc(c),
                            start=(c == 0), stop=(c == 7),
                        )
                    hm = wp.tile([128, T], dt.float16, tag="Hs", name=f"Hs{i}_{q}", bufs=5)
                    nc.scalar.activation(
                        hm[:], ps_h[:], AF.Gelu, bias=b1v[:, q : q + 1]
                    )
                    Hs.append(hm)

                # small psum: per j, cols 0:32 = down-proj t, col 32 = detector z
                ps_small = pp.tile(
                    [128, 4, 36], dt.float32, tag="small", name=f"pss{i}", bufs=2
                )
                for j in range(4):
                    # detector mm2, natural orientation: z[tok,1]
                    for q in range(4):
                        nc.tensor.matmul(
                            ps_small[:, j, 32:33], Hs[q][:, ts(j, 128)],
                            w2v[:, q : q + 1],
                            start=(q == 0), stop=(q == 3),
                        )
                    # down-proj, token-partition orientation: t[tok, 32]
                    for c in range(8):
                        nc.tensor.matmul(
                            ps_small[:, j, 0:32], xtc(c)[:, j, :], dcv[:, c, :],
                            start=(c == 0), stop=False,
                        )
                    nc.tensor.matmul(
                        ps_small[:, j, 0:32], ones1[:, ts(j, 128)], dbrow,
                        start=False, stop=True,
                    )

                # mask = (z > thr) / LAM
                maskn = wp.tile([128, 4, 1], dt.float32, tag="maskn", name=f"mk{i}", bufs=2)
                nc.vector.tensor_scalar(
                    maskn[:], ps_small[:, :, 32:33], thrv, 1.0 / LAM, ALU.is_gt, ALU.mult
                )

                # G^T natural [tok, 36] f16 (cols 32:36 = 1.0 for the up bias)
                GT = wp.tile([128, 4, KUP], dt.float16, tag="GT", name=f"GT{i}", bufs=2)
                nc.gpsimd.memset(GT[:, :, 32:KUP], 1.0)
                for j in range(4):
                    nc.scalar.activation(GT[:, j, 0:32], ps_small[:, j, 0:32], AF.Gelu)

                # transpose G^T -> G [18, 2, j, tok] (DoubleRow lhsT layout)
                Gtp = pp.tile([18, 2, 4, 128], dt.float16, tag="gt", name=f"gtp{i}", bufs=1)
                for j in range(4):
                    for half in range(2):
                        nc.tensor.transpose(
                            Gtp[:, half, j, :], GT[:, j, 18 * half : 18 * half + 18], idh
                        )
                G = wp.tile([18, 2, 4, 128], dt.float8e4, tag="G", name=f"G{i}", bufs=2)
                nc.scalar.copy(G[:], Gtp[:])

                # up-proj (fp8 DoubleRow, natural out) + fused mask+residual
                Obf = wp.tile([128, 4, H], dt.float16, tag="Obf", name=f"Obf{i}", bufs=2)
                for j in range(4):
                    for n in range(2):
                        ps_w = pp.tile(
                            [128, PD], dt.float32, tag="w",
                            name=f"psw{i}_{j}_{n}", bufs=2,
                        )
                        nc.tensor.matmul(
                            ps_w[:], G[:, :, j, :], uw[:, :, ts(n, PD)],
                            start=True, stop=True, perf_mode=PM.DoubleRow,
                        )
                        nc.vector.scalar_tensor_tensor(
                            Obf[:, j, ts(n, PD)], ps_w[:], maskn[:, j, :],
                            Xp[:, j, ts(n, PD)], ALU.mult, ALU.add,
                        )
                    if j % 2 == 1:
                        h = j // 2
                        nc.gpsimd.dma_start(
                            out=out[
                                i * T + h * 256 : i * T + (h + 1) * 256, :
                            ].rearrange("(j p) h -> p j h", p=128),
                            in_=Obf[:, 2 * h : 2 * h + 2, :],
                        )

    nc.compile()
    return nc


_CACHE = {}


def _get_nc():
    if "nc" not in _CACHE:
        _CACHE["nc"] = _build()
    return _CACHE["nc"]


def _host_params(inputs):
    f32 = np.float32
    f16 = np.float16
    pd_w1 = np.asarray(inputs["pd_w1"], f32)          # [H, PD]
    pd_b1 = np.asarray(inputs["pd_b1"], f32)          # [PD]
    pd_w2 = np.asarray(inputs["pd_w2"], f32)          # [PD, 1]
    pd_b2 = np.asarray(inputs["pd_b2"], f32)          # [1]
    down_w = np.asarray(inputs["down_w"], f32)        # [A, H, d]
    down_b = np.asarray(inputs["down_b"], f32)        # [A, d]
    up_w = np.asarray(inputs["up_w"], f32)            # [A, d, H]
    up_b = np.asarray(inputs["up_b"], f32)            # [A, H]
    sel_w = np.asarray(inputs["sel_w"], f32)          # [H, A]
    sel_b = np.asarray(inputs["sel_b"], f32)          # [A]

    # f16 blob: w1 | dcat | w2 | idh | dbrow
    w1s = pd_w1.reshape(8, 128, PD).transpose(1, 0, 2).reshape(128, 8 * PD)
    dcat = down_w.transpose(1, 0, 2).reshape(H, 32)
    dcats = dcat.reshape(8, 128, 32).transpose(1, 0, 2).reshape(128, 256)
    w2s = pd_w2.reshape(4, 128).T
    dbr = np.zeros((128, 32), f32)
    dbr[0, :] = down_b.reshape(32)
    fb16 = np.concatenate([w1s, dcats, w2s, np.eye(128), dbr], axis=1).astype(f16)
    assert fb16.shape == (128, _F16_COLS)

    # f32 blob: b1(4) | thr(1) | selw(32) | selb(1) | o14(4) | i4(4) | e36(36)
    b1s = pd_b1.reshape(4, 128).T
    thr = np.full((128, 1), math.log(THRESH / (1.0 - THRESH)) - float(pd_b2[0]), f32)
    selws = sel_w.reshape(8, 128, 4).transpose(1, 0, 2).reshape(128, 32)
    selbcol = np.zeros((128, 1), f32)
    selbcol[0:4, 0] = sel_b / 2.0
    o14 = np.zeros((128, 4), f32)
    o14[0, :] = 1.0
    i4m = np.zeros((128, 4), f32)
    i4m[0:4, :] = np.eye(4)
    e36m = np.zeros((128, KUP), f32)
    for r in range(32):
        e36m[r // 8, r] = 1.0
    for a in range(4):
        e36m[a, 32 + a] = 1.0
    fb32 = np.concatenate(
        [b1s, thr, selws, selbcol, o14, i4m, e36m], axis=1
    ).astype(f32)
    assert fb32.shape == (128, _F32_COLS)

    # up matrix (36 rows = 32 up_w + 4 up_b) scaled by SCALE*LAM, DoubleRow layout
    u36flat = np.concatenate(
        [SCALE * LAM * up_w.reshape(32, H), SCALE * LAM * up_b], axis=0
    )  # [36, H]
    u36 = u36flat.reshape(2, 18, H).transpose(1, 0, 2).astype(f16)  # [18, 2, H]
    return dict(fb16=fb16, fb32=fb32, u36=u36)


def _run(inputs, trace=False, **kwargs):
    nc = _get_nc()
    params = _host_params(inputs)
    hs = np.asarray(inputs["hidden_states"], np.float32)
    in_maps = [dict(params, x=np.ascontiguousarray(hs[b])) for b in range(N_CORES)]
    try:
        res = run_bass_kernel_spmd(
            nc, in_maps, core_ids=list(range(N_CORES)), trace=trace, **kwargs
        )
    except ModuleNotFoundError:
        res = run_bass_kernel_spmd(
            nc, in_maps, core_ids=list(range(N_CORES)), trace=False, **kwargs
        )
    out = np.stack(
        [np.asarray(res.results[b]["out"]) for b in range(N_CORES)], axis=0
    )
    return out.astype(np.float32), res


def kernel(**inputs) -> np.ndarray:
    out, _ = _run(inputs, trace=False)
    return out
